# revision 1
# baseline (speedup 1.0000x reference)
"""GravityField Trainium2 kernel.

out = U * sqrt(1 + clip(0.1 * grav, -0.9, 5) + 1e-6)
where grav[t] = phi[t] . sum_t'(phi[t'] * mass[t']), phi = sqrt(2/R)*cos(coords@W+b),
mass = softplus(relu(coords@w1+b1)@w2+b2).

Sharding: pure data-parallel over B (8 batches -> 8 cores, no communication).
Each core processes coords [8192, 64] and U [8192, 512] (= 64*8 flattened).

Structure (all fp32): pass 1 computes phiT [R, T] (range-reduced Sin; the
Scalar Engine Sin only accepts [-pi, pi]) and massT [1, T]; phi_sum comes
from a PE ones-broadcast of mass + DVE multiply/reduce against phiT (avoids
per-128-chunk transposes and N=1 matmuls, which dominate fp32 PE time);
pass 2 computes grav in [1, 512] orientation and tiny K=1 transposes back
to per-partition scale columns.
"""

import sys

sys.path.insert(0, "/opt/trn_rl_repo")

import numpy as np
from contextlib import ExitStack

import concourse.bass as bass
import concourse.bacc as bacc
import concourse.mybir as mybir
from concourse import tile
from concourse.bass_utils import run_bass_kernel_spmd
from concourse.masks import make_identity

F32 = mybir.dt.float32
AF = mybir.ActivationFunctionType
ALU = mybir.AluOpType

B, T, D, R_LR, N_RFF = 8, 8192, 64, 8, 64
F = D * R_LR  # 512 floats of U per (b, t)
STRENGTH = 0.1
HALF_PI = 1.5707963267948966
TWO_PI = 6.283185307179586
INV_2PI = 0.15915494309189535
MAGIC = 12582912.0  # 1.5 * 2**23: fp32 add/sub rounds to nearest integer
PI_CLAMP = 3.14159  # strictly inside [-pi, pi] for the ACT Sin table
BIGC = 512
N_BIG = T // BIGC  # 16
CHUNK = 128
N_CHUNKS = T // CHUNK  # 64
PHI_SUM_SCALE = STRENGTH * 2.0 / N_RFF


def build_program():
    nc = bacc.Bacc("TRN2", target_bir_lowering=False, debug=False, num_devices=8)

    u_d = nc.dram_tensor("U", [T, F], F32, kind="ExternalInput")
    coords_d = nc.dram_tensor("coords", [T, D], F32, kind="ExternalInput")
    w1_d = nc.dram_tensor("mass_w1", [D, D], F32, kind="ExternalInput")
    b1_d = nc.dram_tensor("mass_b1", [D], F32, kind="ExternalInput")
    w2_d = nc.dram_tensor("mass_w2", [D, 1], F32, kind="ExternalInput")
    b2_d = nc.dram_tensor("mass_b2", [1], F32, kind="ExternalInput")
    rffw_d = nc.dram_tensor("rff_W", [D, N_RFF], F32, kind="ExternalInput")
    rffb_d = nc.dram_tensor("rff_b", [N_RFF], F32, kind="ExternalInput")
    out_d = nc.dram_tensor("out", [T, F], F32, kind="ExternalOutput")
    mscr_d = nc.dram_tensor("mscr", [N_BIG, BIGC], F32)  # mass broadcast bounce

    with tile.TileContext(nc) as tc, ExitStack() as ctx:
        const = ctx.enter_context(tc.tile_pool(name="const", bufs=1))

        identity = const.tile([128, 128], F32)
        make_identity(nc, identity[:])

        # stationary operands must have one producing engine (PE LW micro-op
        # encodes a single semaphore wait) -> bounce DMA'd weights off DVE
        w_stage = const.tile([65, 128], F32)
        nc.sync.dma_start(w_stage[0:64, 0:64], w1_d[:, :])
        nc.sync.dma_start(w_stage[64:65, 0:64], b1_d[None, :])
        nc.sync.dma_start(w_stage[0:64, 64:128], rffw_d[:, :])
        nc.sync.dma_start(w_stage[64:65, 64:128], rffb_d[None, :])
        nc.vector.tensor_scalar_add(w_stage[64:65, 64:128], w_stage[64:65, 64:128], HALF_PI)
        w_comb = const.tile([65, 128], F32)
        nc.vector.tensor_copy(w_comb[:], w_stage[:])

        w2_stage = const.tile([D, 1], F32)
        nc.sync.dma_start(w2_stage[:], w2_d[:, :])
        w2_sb = const.tile([D, 1], F32)
        nc.vector.tensor_copy(w2_sb[:], w2_stage[:])

        one11 = const.tile([1, 1], F32)
        nc.vector.memset(one11[:], 1.0)
        ones1_64 = const.tile([1, N_RFF], F32)
        nc.vector.memset(ones1_64[:], 1.0)
        b2_sb = const.tile([1, 1], F32)
        nc.sync.dma_start(b2_sb[:], b2_d[None, :])
        b2_neg_sb = const.tile([1, 1], F32)
        nc.vector.tensor_scalar_mul(b2_neg_sb[:], b2_sb[:], -1.0)
        sqrt_bias = const.tile([128, 1], F32)
        nc.vector.memset(sqrt_bias[:], 1.000001)
        phi_sum = const.tile([N_RFF, 1], F32)

        phiT_all = const.tile([N_RFF, T], F32)   # cos features, [R, T]
        massT_all = const.tile([1, T], F32)      # -mass pre-act then -mass, [1, T]
        partials = const.tile([N_RFF, N_BIG], F32)

        coords_pool = ctx.enter_context(tc.tile_pool(name="coords", bufs=3))
        caug_pool = ctx.enter_context(tc.tile_pool(name="caug", bufs=2))
        hT_pool = ctx.enter_context(tc.tile_pool(name="hT", bufs=2))
        rr_pool = ctx.enter_context(tc.tile_pool(name="rr", bufs=2))
        phw_pool = ctx.enter_context(tc.tile_pool(name="phw", bufs=2))
        bc_pool = ctx.enter_context(tc.tile_pool(name="bc", bufs=2))
        u_pool = ctx.enter_context(tc.tile_pool(name="u", bufs=55))
        scale_pool = ctx.enter_context(tc.tile_pool(name="scale", bufs=4))

        u_tiles = []

        with (
            tc.tile_pool(name="ptr", bufs=2, space=bass.MemorySpace.PSUM) as ptr_pool,
            tc.tile_pool(name="pbig", bufs=2, space=bass.MemorySpace.PSUM) as pbig_pool,
            tc.tile_pool(name="pmT", bufs=2, space=bass.MemorySpace.PSUM) as pmT_pool,
        ):
            for c in range(N_BIG):
                tsl = slice(c * BIGC, (c + 1) * BIGC)

                ct = coords_pool.tile([128, 4 * D], F32, tag="ct")
                src = coords_d[tsl, :].rearrange("(j p) d -> p j d", p=128)
                nc.gpsimd.dma_start(ct[:].rearrange("p (j d) -> p j d", j=4), src)

                tp = ptr_pool.tile([D, BIGC], F32, tag="tp")
                for j in range(4):
                    nc.tensor.transpose(
                        tp[:, j * 128 : (j + 1) * 128],
                        ct[:, j * D : (j + 1) * D],
                        identity[:],
                    )
                caug = caug_pool.tile([D + 1, BIGC], F32, tag="caug")
                nc.vector.tensor_copy(caug[0:D, :], tp[:])
                nc.vector.memset(caug[D : D + 1, :], 1.0)

                big = pbig_pool.tile([128, BIGC], F32, tag="big")
                nc.tensor.matmul(big[:], w_comb[:], caug[:], start=True, stop=True)

                hT = hT_pool.tile([D, BIGC], F32, tag="hT")
                nc.vector.tensor_scalar_max(hT[:], big[0:D, :], 0.0)  # relu

                # mass pre-act in [1, 512] orientation: trivial weight load
                mT = pmT_pool.tile([1, BIGC], F32, tag="mT")
                nc.tensor.matmul(mT[:], w2_sb[:], hT[:], start=True, stop=True)
                # -mass = ln(sigmoid(-(pre + b2)))
                nc.scalar.activation(
                    massT_all[:, tsl], mT[:], AF.Sigmoid, bias=b2_neg_sb[:], scale=-1.0
                )
                nc.scalar.activation(massT_all[:, tsl], massT_all[:, tsl], AF.Ln)
                # phi_sum partial: broadcast mass to [R, 512] via a 0-stride
                # DRAM re-read (off-PE), then DVE mul + reduce
                nc.sync.dma_start(mscr_d[c : c + 1, :], massT_all[:, tsl])
                bc = bc_pool.tile([N_RFF, BIGC], F32, tag="bc")
                nc.sync.dma_start(bc[:], mscr_d[c : c + 1, :].to_broadcast((N_RFF, BIGC)))

                # range-reduce x -> [-pi, pi]: y = x - 2pi*round(x/2pi)
                x = big[D : 2 * D, :]
                tmp = rr_pool.tile([D, BIGC], F32, tag="tmp")
                nc.vector.tensor_scalar(
                    tmp[:], x, INV_2PI, MAGIC, op0=ALU.mult, op1=ALU.add
                )
                nc.vector.tensor_scalar(
                    tmp[:], tmp[:], MAGIC, -TWO_PI, op0=ALU.subtract, op1=ALU.mult
                )
                nc.vector.tensor_tensor(tmp[:], x, tmp[:], op=ALU.add)
                nc.vector.tensor_scalar(
                    tmp[:], tmp[:], PI_CLAMP, -PI_CLAMP, op0=ALU.min, op1=ALU.max
                )
                nc.scalar.activation(phiT_all[:, tsl], tmp[:], AF.Sin)

                phw = phw_pool.tile([N_RFF, BIGC], F32, tag="phw")
                nc.vector.tensor_tensor(phw[:], phiT_all[:, tsl], bc[:], op=ALU.mult)
                nc.vector.reduce_sum(partials[:, c : c + 1], phw[:], axis=mybir.AxisListType.X)

                for j in range(4):
                    usl = slice(c * BIGC + j * 128, c * BIGC + (j + 1) * 128)
                    ut = u_pool.tile([CHUNK, F], F32, tag="u")
                    nc.sync.dma_start(ut[:], u_d[usl, :])
                    u_tiles.append(ut)

            acc_raw = const.tile([N_RFF, 1], F32)
            nc.vector.reduce_sum(acc_raw[:], partials[:], axis=mybir.AxisListType.X)
            # massT holds -mass -> negate the fold-in scale
            nc.scalar.mul(phi_sum[:], acc_raw[:], -PHI_SUM_SCALE)

        with (
            tc.tile_pool(name="pgT", bufs=2, space=bass.MemorySpace.PSUM) as pgT_pool,
            tc.tile_pool(name="pg4", bufs=2, space=bass.MemorySpace.PSUM) as pg4_pool,
        ):
            for g in range(N_BIG):
                tsl = slice(g * BIGC, (g + 1) * BIGC)
                gT = pgT_pool.tile([1, BIGC], F32, tag="gT")
                # influence in [1, 512] orientation (scales folded into phi_sum)
                nc.tensor.matmul(gT[:], phi_sum[:], phiT_all[:, tsl], start=True, stop=True)
                gsb = scale_pool.tile([1, BIGC], F32, tag="gsb")
                nc.vector.tensor_scalar(
                    gsb[:], gT[:], -0.9, 5.0, op0=ALU.max, op1=ALU.min
                )
                pg4 = pg4_pool.tile([128, 4], F32, tag="pg4")
                for j in range(4):
                    # K=1 matmul = transpose [1,128] -> [128,1]
                    nc.tensor.matmul(
                        pg4[:, j : j + 1],
                        gsb[0:1, j * 128 : (j + 1) * 128],
                        one11[:],
                        start=True,
                        stop=True,
                    )
                sc4 = scale_pool.tile([128, 4], F32, tag="sc4")
                nc.scalar.activation(sc4[:], pg4[:], AF.Sqrt, bias=sqrt_bias[:])

                for j in range(4):
                    c = 4 * g + j
                    tslU = slice(c * CHUNK, (c + 1) * CHUNK)
                    ut = u_tiles[c]
                    if c % 2 == 0:
                        nc.vector.tensor_scalar_mul(ut[:], ut[:], sc4[:, j : j + 1])
                    else:
                        nc.scalar.mul(ut[:], ut[:], sc4[:, j : j + 1])
                    nc.sync.dma_start(out_d[tslU, :], ut[:])

    nc.compile()
    return nc


_NC_CACHE = None


def _get_program():
    global _NC_CACHE
    if _NC_CACHE is None:
        _NC_CACHE = build_program()
    return _NC_CACHE


def run(inputs: dict, trace: bool = False, tmpdir=None):
    nc = _get_program()
    U = np.ascontiguousarray(np.asarray(inputs["U"], dtype=np.float32)).reshape(B, T, F)
    coords = np.ascontiguousarray(np.asarray(inputs["coords"], dtype=np.float32))
    shared = {
        "mass_w1": np.ascontiguousarray(np.asarray(inputs["mass_w1"], np.float32)),
        "mass_b1": np.ascontiguousarray(np.asarray(inputs["mass_b1"], np.float32)),
        "mass_w2": np.ascontiguousarray(np.asarray(inputs["mass_w2"], np.float32)),
        "mass_b2": np.ascontiguousarray(np.asarray(inputs["mass_b2"], np.float32)),
        "rff_W": np.ascontiguousarray(np.asarray(inputs["rff_W"], np.float32)),
        "rff_b": np.ascontiguousarray(np.asarray(inputs["rff_b"], np.float32)),
    }
    in_maps = [{"U": U[i], "coords": coords[i], **shared} for i in range(B)]
    res = run_bass_kernel_spmd(nc, in_maps, list(range(B)), trace=trace, tmpdir=tmpdir)
    out = np.stack([res.results[i]["out"].reshape(T, D, R_LR) for i in range(B)])
    return out.astype(np.float32), res


def kernel(**inputs) -> np.ndarray:
    out, _ = run(inputs, trace=False)
    return out

